# revision 37
# baseline (speedup 1.0000x reference)
"""GQA attention (B=2, L=2048, D=2048, Hq=32, Hkv=8, hd=64) on 8 TRN2 cores.

Tensor-parallel over heads: core c owns q heads 4c..4c+3 and kv head c.
Each core computes a partial output (wo input-dim shard); host sums partials.

Design (all-bf16 data path; fp8 was tested and rejected: quantizing any of
q/k/e/v costs 2.5-3.6e-2 rel err because peaked softmax rows do not average
quantization noise away):
  * bf16 x/weights/q/k/v/e/atP: 1 cycle/row matmuls, half the DMA, and
    DVE 2x/4x vector modes; f32 only inside PSUM accumulation
  * K+V projections share one 128-col stationary; V is PE-transposed to
    token-major (frees PSUM banks -> double-buffered projection psums)
  * host pre-tiles all DRAM inputs partition-major -> single contiguous DMAs
  * score chunks processed in [128, 2, NB] pair tiles (2 PSUM banks, two
    accumulation groups); even/odd heads interleaved so adjacent score
    matmuls land in different PE row groups (K=64) and overlap in the array
  * diagonal chunks skip the fully-masked q-column prefix in the score
    matmul, exp, and attn@V matmul; causal boundary via one bf16 triangle
    multiply (DVE 4x mode)
  * softmax denominator via ones-column in the V stationary (no-max exp);
    one ACT eviction of the whole [65, NB] accumulator frees the PSUM bank
    immediately (the next head-pair's accumulation starts ~2.5us earlier);
    normalization runs lazily from SBUF: reciprocal_approx_fast (needs a
    partition-0 SBUF operand -- it misreads both PSUM and offset-base
    inputs) + Pool partition_broadcast + one DVE multiply
  * out-projection in mc pairs sharing the score psum pool; bf16 partial
    output, DMA dispatch split across the SP and Pool (software DGE) queues
  * each block's out-projection is emission-deferred and drained one mc-pair
    per score-pair step of the NEXT block, so the in-order PE queue never
    stalls behind the normalization chain at block boundaries

Per-core DRAM layouts (host prepped, partition-major):
  xTt     [128, NT*KC*NB]  x transposed/tiled, bf16 (same for all cores)
  wq_t    [128, KC*DQ]     wq shard, per-head [even|odd] row perm, bf16
  wkv_t   [128, KC*128]    [wk shard (perm) | wv shard], bf16
  wo_t    [128, 2*D]       wo column shard, bf16
  outT    [128, NT, 16, NB] partial output, bf16 (host: sum cores, reorder)
"""
import ml_dtypes
import numpy as np
from contextlib import ExitStack

import concourse.bass as bass
import concourse.mybir as mybir
import concourse.tile as tile
from concourse import bacc
from concourse.bass_utils import run_bass_kernel_spmd

F32 = mybir.dt.float32
F32R = mybir.dt.float32r
BF16 = mybir.dt.bfloat16
FP8 = mybir.dt.float8e4
U8 = mybir.dt.uint8
U32 = mybir.dt.uint32
I32 = mybir.dt.int32
AF = mybir.ActivationFunctionType
ALU = mybir.AluOpType

B, L, D = 2, 2048, 2048
HQ, HKV, HD = 32, 8, 64
NCORES = 8
HL = HQ // NCORES          # 4 q heads per core
DQ = HL * HD               # 256 local q features
T = B * L                  # 4096 tokens
NB = 512                   # token block
NT = T // NB               # 8 token blocks
KC = D // 128              # 16 contraction chunks
ROPE_BASE = 10000.0
SCALE = 1.0 / np.sqrt(HD)

_CACHE = {}


def _build_module():
    nc = bacc.Bacc("TRN2", target_bir_lowering=False, debug=False,
                   num_devices=NCORES)

    # host pre-tiles to partition-major [128, ...] so loads are single
    # contiguous 2D DMAs
    d_xT = nc.dram_tensor("xTt", [128, NT * KC * NB], BF16,
                          kind="ExternalInput").ap()
    d_wq = nc.dram_tensor("wq_t", [128, KC * DQ], BF16, kind="ExternalInput").ap()
    d_wkv = nc.dram_tensor("wkv_t", [128, KC * 128], BF16, kind="ExternalInput").ap()
    d_wo = nc.dram_tensor("wo_t", [128, 2 * D], BF16, kind="ExternalInput").ap()
    d_pos = nc.dram_tensor("pos", [1, L], I32, kind="ExternalInput").ap()
    d_invf = nc.dram_tensor("invf", [128, 1], F32, kind="ExternalInput").ap()
    d_ones64 = nc.dram_tensor("ones64", [1, 64], F32R, kind="ExternalInput").ap()
    d_ident = nc.dram_tensor("ident64", [64, 64], BF16, kind="ExternalInput").ap()
    d_tri = nc.dram_tensor("triw", [128, 128], BF16, kind="ExternalInput").ap()
    d_out = nc.dram_tensor("outT", [128, NT, 16, NB], BF16,
                           kind="ExternalOutput").ap()

    with tile.TileContext(nc) as tc, ExitStack() as ctx, \
         nc.allow_low_precision(reason="bf16 matmul pipeline"):
        _kernel(tc, ctx, d_xT, d_wq, d_wkv, d_wo, d_pos, d_invf,
                d_ones64, d_ident, d_tri, d_out)

    nc.compile()
    return nc


def _kernel(tc, ctx, d_xT, d_wq, d_wkv, d_wo, d_pos, d_invf,
            d_ones64, d_ident, d_tri, d_out):
    nc = tc.nc

    wpool = ctx.enter_context(tc.tile_pool(name="weights", bufs=1))
    spool = ctx.enter_context(tc.tile_pool(name="state", bufs=1))

    # ---------------- persistent SBUF tensors ----------------
    wqT = wpool.tile([128, KC * DQ], BF16, tag="wqT")      # 8KB/part
    wkvT = wpool.tile([128, KC * 128], BF16, tag="wkvT")   # 4KB
    woT = wpool.tile([128, 2 * D], BF16, tag="woT")        # 8KB
    nc.sync.dma_start(wqT[:], d_wq[:])
    nc.sync.dma_start(wkvT[:], d_wkv[:])
    nc.sync.dma_start(woT[:], d_wo[:])

    ones64 = wpool.tile([1, 64], F32R, tag="ones64")
    nc.sync.dma_start(ones64[:], d_ones64[:])
    ident64 = wpool.tile([64, 64], BF16, tag="ident64")
    nc.sync.dma_start(ident64[:], d_ident[:])

    # qT: [128, HL/2 * T]; head pair p on partitions (even head rows 0:64,
    # odd head rows 64:128); within a head [even dims | odd dims].
    qT = spool.tile([128, 2, T], BF16, tag="qT")           # 16KB
    # kT duplicated on partitions 64:128 so odd-head matmuls get equal bases.
    kT = spool.tile([128, T], BF16, tag="kT")              # 8KB
    # v token-major bf16, chunk-pair layout: [p, pair, par, 80]
    # col 64 of each chunk = ones (softmax denominator); cols 65:80 pad
    vA = spool.tile([128, 16, 2, 80], BF16, tag="vA")      # 5KB
    nc.gpsimd.memset(vA[:, :, :, :], 1.0)  # ones cols; data cols overwritten
    # attention output, transposed: head pair tiles, b-major columns
    atP = [spool.tile([128, T], BF16, tag=f"atP{p}", name=f"atP{p}") for p in range(2)]  # 16KB

    c128 = spool.tile([128, L], BF16, tag="c128")          # 4KB
    s128 = spool.tile([128, L], BF16, tag="s128")          # 4KB
    # causal triangle window (multiplicative bf16): keep where col >= part
    triW = spool.tile([128, 128], BF16, tag="triW")
    nc.sync.dma_start(triW[:], d_tri[:])

    def build_trig_masks():
        # ---------------- trig tables (cos/sin on 128 partitions) -------------
        # rows 0:32 / 32:64 / 64:96 / 96:128 all hold the same [32] invfreq set,
        # so c128/s128 serve every 32-row band.
        with tc.tile_pool(name="trig", bufs=1) as trig:
            # 5 shared 8KB slots (tags A..E) for all [*, L]-sized temporaries
            pos_i = trig.tile([1, L], I32, tag="sA", name="pos_i")
            nc.sync.dma_start(pos_i[:], d_pos[:])
            pos_f = trig.tile([1, L], F32, tag="sB", name="pos_f")
            nc.vector.tensor_copy(pos_f[:], pos_i[:])
            posb = trig.tile([128, L], F32, tag="sC", name="posb")
            nc.gpsimd.partition_broadcast(posb[:], pos_f[:])
            invf = trig.tile([128, 1], F32, tag="invf")
            nc.sync.dma_start(invf[:], d_invf[:])
            fq = trig.tile([128, L], F32, tag="sD", name="fq")
            nc.vector.tensor_scalar(fq[:], posb[:], invf[:], None, ALU.mult)
            # Cody-Waite range reduction, k via magic-number round-to-nearest
            INV2PI = float(np.float32(1.0 / (2 * np.pi)))
            C1 = float(np.float32(6.28125))
            C2 = float(np.float32(0.0019353071795864769))
            MAGIC = float(np.float32(12582912.0))              # 1.5 * 2**23
            t_a = trig.tile([128, L], F32, tag="sE", name="t_a")
            nc.vector.tensor_scalar(t_a[:], fq[:], INV2PI, None, ALU.mult)
            t_b = trig.tile([128, L], F32, tag="sC", name="t_b")
            nc.vector.tensor_scalar(t_b[:], t_a[:], MAGIC, None, ALU.add)
            kk = trig.tile([128, L], F32, tag="sB", name="kk")
            nc.vector.tensor_scalar(kk[:], t_b[:], MAGIC, None, ALU.subtract)
            nc.vector.tensor_scalar(t_a[:], kk[:], C1, None, ALU.mult)
            nc.vector.tensor_sub(fq[:], fq[:], t_a[:])
            nc.vector.tensor_scalar(t_a[:], kk[:], C2, None, ALU.mult)
            nc.vector.tensor_sub(fq[:], fq[:], t_a[:])         # fq = reduced angle r
            sf = trig.tile([128, L], F32, tag="sB", name="sf")
            nc.scalar.activation(sf[:], fq[:], AF.Sin)
            nc.scalar.activation(t_a[:], fq[:], AF.Abs)
            pi2 = trig.tile([128, 1], F32, tag="pi2")
            nc.gpsimd.memset(pi2[:], float(np.pi / 2))
            cf = trig.tile([128, L], F32, tag="sD", name="cf")
            nc.scalar.activation(cf[:], t_a[:], AF.Sin, bias=pi2[:], scale=-1.0)
            # bake rotation signs into sin table: even-dim bands get -sin
            for band in (0, 2):
                nc.vector.tensor_scalar(sf[band * 32:(band + 1) * 32, :],
                                        sf[band * 32:(band + 1) * 32, :],
                                        -1.0, None, ALU.mult)
            nc.vector.tensor_copy(c128[:], cf[:])
            nc.vector.tensor_copy(s128[:], sf[:])


    # ---------------- phase 1.5: RoPE -------------------------------------
    # per (head-pair, b): bands {ev, od, ev, od} on partitions
    tpool = ctx.enter_context(tc.tile_pool(name="tmp", bufs=1))

    def rope(view, l0):
        # y = x*cos + swap(x)*sgn*sin, swap = exchange 32-row halves per head
        # view(r0, r1) must return the [r0:r1, NB] slice of the target
        u = tpool.tile([128, NB], BF16, tag="ropeU")
        w = tpool.tile([128, NB], BF16, tag="ropeW")
        xsw = tpool.tile([128, NB], BF16, tag="ropeX")
        for band in range(4):
            r0, r1 = band * 32, (band ^ 1) * 32
            nc.vector.tensor_copy(xsw[r0:r0 + 32, :], view(r1, r1 + 32))
        nc.vector.tensor_mul(u[:], view(0, 128), c128[:, l0:l0 + NB])
        nc.vector.tensor_mul(w[:], xsw[:], s128[:, l0:l0 + NB])
        nc.vector.tensor_add(view(0, 128), u[:], w[:])

    def rope_nt(nt):
        b, l0 = nt // 4, (nt % 4) * NB
        c0 = b * L + l0
        for p in range(2):
            rope(lambda r0, r1, p=p: qT[r0:r1, p, c0:c0 + NB], l0)
        rope(lambda r0, r1: kT[r0:r1, c0:c0 + NB], l0)

    # ---------------- phase 1: QKV projections ----------------------------
    with tc.tile_pool(name="xs", bufs=4) as xs, \
         tc.tile_pool(name="vf", bufs=2) as vfp, \
         tc.tile_pool(name="pproj", bufs=2, space="PSUM") as pq, \
         tc.tile_pool(name="pprojkv", bufs=2, space="PSUM") as pkv, \
         tc.tile_pool(name="ptr", bufs=2, space="PSUM") as ptr:
        trig_done = False
        pending_rope = []
        for nt in [0, 1, "trig", 2, 3, 4, 5, 6, 7]:
            if nt == "trig":
                build_trig_masks()
                trig_done = True
                for pnt in pending_rope:
                    rope_nt(pnt)
                pending_rope.clear()
                continue
            # one big DMA for this token block: [128, KC*NB] contiguous
            xk = xs.tile([128, KC * NB], BF16, tag="xk", name=f"xk{nt}")
            nc.sync.dma_start(
                xk[:], d_xT[:, nt * KC * NB:(nt + 1) * KC * NB])
            psq = pq.tile([128, 2, NB], F32, tag="psq", name=f"psq{nt}")
            pskv = pkv.tile([128, NB], F32, tag="pskv", name=f"pskv{nt}")
            for kc in range(KC):
                xkc = xk[:, kc * NB:(kc + 1) * NB]
                st, sp = kc == 0, kc == KC - 1
                for p in range(2):
                    nc.tensor.matmul(
                        psq[:, p, :], wqT[:, kc * DQ + p * 128: kc * DQ + (p + 1) * 128],
                        xkc, start=st, stop=sp, skip_group_check=True)
                nc.tensor.matmul(pskv[:], wkvT[:, kc * 128:(kc + 1) * 128], xkc,
                                 start=st, stop=sp)
            # evictions (ACT copies round fp32 -> bf16); q pair in one op
            nc.scalar.copy(qT[:, :, nt * NB:(nt + 1) * NB], psq[:, :, :])
            nc.scalar.copy(kT[0:64, nt * NB:(nt + 1) * NB], pskv[0:64, :])
            nc.vector.tensor_copy(kT[64:128, nt * NB:(nt + 1) * NB], pskv[0:64, :])
            # V: evict feature-major slab, then PE-transpose to token-major
            vf = vfp.tile([64, NB], BF16, tag="vf", name=f"vf{nt}")
            nc.scalar.copy(vf[:], pskv[64:128, :])
            for c4 in range(4):
                ch = nt * 4 + c4
                psT = ptr.tile([128, 64], BF16, tag="psT", name=f"psT{ch}",
                               padded_shape=[128, 1024])
                nc.tensor.transpose(psT[:], vf[:, c4 * 128:(c4 + 1) * 128],
                                    ident64[:])
                nc.scalar.copy(vA[:, ch // 2, ch % 2, 0:64], psT[:])
            if trig_done:
                rope_nt(nt)
            else:
                pending_rope.append(nt)

    # ---------------- phase 2+3: attention + out-projection ---------------
    # Score chunks processed in PAIRS: one [128, 2, NB] PSUM tile (2 banks),
    # one batched exp -> fp8, one DoubleRow matmul contracting both chunks.
    # pst pool (3 bufs x 2 banks) is shared with the out-projection psums.
    with tc.tile_pool(name="epool", bufs=8) as ep, \
         tc.tile_pool(name="npool", bufs=3) as npool, \
         tc.tile_pool(name="opool", bufs=6) as op, \
         tc.tile_pool(name="pst", bufs=3, space="PSUM") as pst, \
         tc.tile_pool(name="pot", bufs=2, space="PSUM") as pot:
        pending = []  # deferred out-projection emissions (prev block)

        def emit_oproj_pair(nt, mcp):
            po = pst.tile([128, 2, NB], F32, tag="st", name=f"po{nt}_{mcp}")
            for m in range(2):
                mc = 2 * mcp + m
                nc.tensor.matmul(po[:, m, :],
                                 woT[:, 0 * D + mc * 128: 0 * D + (mc + 1) * 128],
                                 atP[0][:, nt * NB:(nt + 1) * NB],
                                 start=True, stop=False, skip_group_check=True)
                nc.tensor.matmul(po[:, m, :],
                                 woT[:, 1 * D + mc * 128: 1 * D + (mc + 1) * 128],
                                 atP[1][:, nt * NB:(nt + 1) * NB],
                                 start=False, stop=True, skip_group_check=True)
            osb = op.tile([128, 2, NB], BF16, tag="osb")
            nc.vector.tensor_copy(osb[:, :, :], po[:, :, :])
            dma_eng = nc.sync if mcp % 2 == 0 else nc.gpsimd
            dma_eng.dma_start(d_out[:, nt, 2 * mcp:2 * mcp + 2, :],
                              osb[:, :, :])

        for b in range(B):
            for ib in range(L // NB):
                # heads interleaved in pairs (even rows 0:64 / odd rows 64:128)
                # so adjacent score matmuls land in different PE row groups
                for p in range(2):
                    qc0 = b * L + ib * NB
                    ot2 = [pot.tile([65, NB], F32, tag="ot", name=f"ot{p}_{o}")
                           for o in range(2)]
                    npc = 2 * (ib + 1)  # chunk pairs
                    for pc in range(npc):
                        st4 = [pst.tile([128, 2, NB], F32, tag="st",
                                        name=f"st{p}_{pc}_{o}") for o in range(2)]
                        for par in range(2):
                            jc = 2 * pc + par
                            c = jc - 4 * ib  # >=0 on diagonal chunks
                            w0 = max(c, 0) * 128  # skip fully-masked q prefix
                            for odd in range(2):
                                rbase = 64 * odd
                                nc.tensor.matmul(
                                    st4[odd][:, par, w0:],
                                    kT[rbase:rbase + 64, b * L + jc * 128: b * L + (jc + 1) * 128],
                                    qT[rbase:rbase + 64, p, qc0 + w0:qc0 + NB],
                                    start=True, stop=True, skip_group_check=True)
                        for odd in range(2):
                            jc0 = 2 * pc
                            cA = jc0 - 4 * ib
                            e_t = ep.tile([128, 2, NB], BF16, tag="e",
                                          name=f"e{p}_{pc}_{odd}")
                            if cA < 0:  # both chunks fully off-diagonal
                                nc.scalar.activation(e_t[:, :, :],
                                                     st4[odd][:, :, :], AF.Exp,
                                                     scale=float(SCALE))
                            else:  # per-chunk exp, skipping masked prefix
                                for par in range(2):
                                    w0 = (cA + par) * 128
                                    nc.scalar.activation(
                                        e_t[:, par, w0:], st4[odd][:, par, w0:],
                                        AF.Exp, scale=float(SCALE))
                            for par in range(2):
                                jc = 2 * pc + par
                                if jc >= 4 * ib:  # triangle window mask
                                    w0 = (jc - 4 * ib) * 128
                                    nc.vector.tensor_mul(
                                        e_t[:, par, w0:w0 + 128],
                                        e_t[:, par, w0:w0 + 128], triW[:])
                            for par in range(2):
                                jc = 2 * pc + par
                                c = jc - 4 * ib
                                w0 = max(c, 0) * 128
                                ch = b * 16 + jc
                                nc.tensor.matmul(
                                    ot2[odd][:, w0:],
                                    vA[:, ch // 2, ch % 2, 0:65],
                                    e_t[:, par, w0:],
                                    start=(jc == 0), stop=(jc == 4 * ib + 3),
                                    skip_group_check=True)
                        # interleave deferred out-proj pairs; short blocks
                        # (npc==2) drain double so no backlog carries over
                        for _ in range(2 if npc == 2 else 1):
                            if pending:
                                pending.pop(0)()
                    # normalization: denom row -> SBUF (recip_approx misreads
                    # PSUM) -> fast recip -> partition-broadcast on Pool -> scale
                    for odd in range(2):
                        rbase = 64 * odd
                        ot_ps = ot2[odd]
                        # single-op eviction frees the psum bank immediately;
                        # the normalization chain then runs lazily from SBUF
                        ot_sb = npool.tile([65, NB], F32, tag="ot_sb")
                        nc.scalar.copy(ot_sb[:], ot_ps[:])
                        # recip_approx needs a partition-0 SBUF operand
                        dns = npool.tile([1, NB], F32, tag="dns")
                        nc.vector.tensor_copy(dns[:], ot_sb[64:65, :])
                        dnr = npool.tile([1, NB], F32, tag="dnr")
                        nc.vector.reciprocal_approx_fast(dnr[:], dns[:])
                        denbf = npool.tile([64, NB], F32, tag="denbf")
                        nc.gpsimd.partition_broadcast(denbf[:], dnr[:])
                        nc.vector.tensor_mul(
                            atP[p][rbase:rbase + 64, b * L + ib * NB: b * L + (ib + 1) * NB],
                            ot_sb[0:64, :], denbf[:])
                # defer this block's out-projection into the next
                # block's score stream (tail-flushed after the last block)
                nt = b * 4 + ib
                for mcp in range(8):
                    pending.append(
                        lambda nt=nt, mcp=mcp: emit_oproj_pair(nt, mcp))
        for f in pending:  # flush the final block's out-projection
            f()

def _deinterleave_rows(w):
    # [H*64, D] -> per-head rows reordered to [even dims | odd dims]
    h = w.shape[0] // HD
    out = np.empty_like(w)
    perm = np.concatenate([np.arange(0, HD, 2), np.arange(1, HD, 2)])
    for i in range(h):
        out[i * HD:(i + 1) * HD] = w[i * HD:(i + 1) * HD][perm]
    return out


def _part_major(wT, nchunk, m):
    # [nchunk*128, m] -> [128, nchunk*m]: row kc*128+p, col j -> [p, kc*m+j]
    return np.ascontiguousarray(
        wT.reshape(nchunk, 128, m).transpose(1, 0, 2).reshape(128, nchunk * m))


def _prep_inputs(x, pos_ids, wq, wk, wv, wo):
    xT = x.reshape(T, D).T.astype(ml_dtypes.bfloat16)        # [D, T]
    # [128, nt*KC*NB]: block nt = chunks kc of [128, NB]
    xTt = np.ascontiguousarray(
        xT.reshape(KC, 128, NT, NB).transpose(1, 2, 0, 3).reshape(128, -1))
    pos = np.ascontiguousarray(pos_ids.astype(np.int32).reshape(1, L))
    half = HD // 2
    invf = (1.0 / (ROPE_BASE ** (np.arange(half, dtype=np.float32) / half)))
    invf128 = np.ascontiguousarray(np.tile(invf, 4).reshape(128, 1))
    ones64 = np.ones((1, 64), np.float32)
    ident64 = np.eye(64, dtype=ml_dtypes.bfloat16)
    triw = ((np.arange(128)[None, :] >= np.arange(128)[:, None])
            .astype(ml_dtypes.bfloat16))
    in_maps = []
    for c in range(NCORES):
        wq_c = _deinterleave_rows(wq[c * DQ:(c + 1) * DQ])
        wk_c = _deinterleave_rows(wk[c * HD:(c + 1) * HD])
        wv_c = wv[c * HD:(c + 1) * HD]
        wkv_c = np.concatenate([wk_c, wv_c], axis=0)      # [128, D]
        wo_c = wo[:, c * DQ:(c + 1) * DQ]
        in_maps.append({
            "xTt": xTt,
            "wq_t": _part_major(wq_c.T.astype(ml_dtypes.bfloat16), KC, DQ),
            "wkv_t": _part_major(wkv_c.T.astype(ml_dtypes.bfloat16), KC, 128),
            "wo_t": _part_major(wo_c.T.astype(ml_dtypes.bfloat16), 2, D),
            "pos": pos,
            "invf": invf128,
            "ones64": ones64,
            "ident64": ident64,
            "triw": triw,
        })
    return in_maps


def kernel(x, pos_ids, wq, wk, wv, wo, _trace=False):
    x = np.asarray(x)
    if "nc" not in _CACHE:
        _CACHE["nc"] = _build_module()
    nc = _CACHE["nc"]
    in_maps = _prep_inputs(np.asarray(x, np.float32), np.asarray(pos_ids),
                           np.asarray(wq, np.float32), np.asarray(wk, np.float32),
                           np.asarray(wv, np.float32), np.asarray(wo, np.float32))
    res = run_bass_kernel_spmd(nc, in_maps, core_ids=list(range(NCORES)),
                               trace=_trace)
    _CACHE["last_results"] = res
    acc = np.zeros((128, NT, 16, NB), np.float32)
    for r in res.results:
        acc += r["outT"].astype(np.float32)
    # [p, nt, mc, n] -> [ mc*128+p, nt*NB+n ]
    outT = acc.transpose(2, 0, 1, 3).reshape(D, T)
    return np.ascontiguousarray(outT.T).reshape(B, L, D)


# revision 38
# speedup vs baseline: 1.0195x; 1.0195x over previous
"""GQA attention (B=2, L=2048, D=2048, Hq=32, Hkv=8, hd=64) on 8 TRN2 cores.

Tensor-parallel over heads: core c owns q heads 4c..4c+3 and kv head c.
Each core computes a partial output (wo input-dim shard); host sums partials.

Design (all-bf16 data path; fp8 was tested and rejected: quantizing any of
q/k/e/v costs 2.5-3.6e-2 rel err because peaked softmax rows do not average
quantization noise away):
  * bf16 x/weights/q/k/v/e/atP: 1 cycle/row matmuls, half the DMA, and
    DVE 2x/4x vector modes; f32 only inside PSUM accumulation
  * K+V projections share one 128-col stationary; V is PE-transposed to
    token-major (frees PSUM banks -> double-buffered projection psums)
  * host pre-tiles all DRAM inputs partition-major -> single contiguous DMAs
  * score chunks processed in [128, 2, NB] pair tiles (2 PSUM banks, two
    accumulation groups); even/odd heads interleaved so adjacent score
    matmuls land in different PE row groups (K=64) and overlap in the array
  * diagonal chunks skip the fully-masked q-column prefix in the score
    matmul, exp, and attn@V matmul; causal boundary via one bf16 triangle
    multiply (DVE 4x mode)
  * softmax denominator via ones-column in the V stationary (no-max exp);
    one ACT eviction of the whole [65, NB] accumulator frees the PSUM bank
    immediately (the next head-pair's accumulation starts ~2.5us earlier);
    normalization runs lazily from SBUF: reciprocal_approx_fast (needs a
    partition-0 SBUF operand -- it misreads both PSUM and offset-base
    inputs) + Pool partition_broadcast + one DVE multiply
  * out-projection in mc pairs sharing the score psum pool; bf16 partial
    output, DMA dispatch split across the SP and Pool (software DGE) queues
  * each block's out-projection is emission-deferred and drained one mc-pair
    per score-pair step of the NEXT block, so the in-order PE queue never
    stalls behind the normalization chain at block boundaries

Per-core DRAM layouts (host prepped, partition-major):
  xTt     [128, NT*KC*NB]  x transposed/tiled, bf16 (same for all cores)
  wq_t    [128, KC*DQ]     wq shard, per-head [even|odd] row perm, bf16
  wkv_t   [128, KC*128]    [wk shard (perm) | wv shard], bf16
  wo_t    [128, 2*D]       wo column shard, bf16
  outT    [128, NT, 16, NB] partial output, bf16 (host: sum cores, reorder)
"""
import ml_dtypes
import numpy as np
from contextlib import ExitStack

import concourse.bass as bass
import concourse.mybir as mybir
import concourse.tile as tile
from concourse import bacc
from concourse.bass_utils import run_bass_kernel_spmd

F32 = mybir.dt.float32
F32R = mybir.dt.float32r
BF16 = mybir.dt.bfloat16
FP8 = mybir.dt.float8e4
U8 = mybir.dt.uint8
U32 = mybir.dt.uint32
I32 = mybir.dt.int32
AF = mybir.ActivationFunctionType
ALU = mybir.AluOpType

B, L, D = 2, 2048, 2048
HQ, HKV, HD = 32, 8, 64
NCORES = 8
HL = HQ // NCORES          # 4 q heads per core
DQ = HL * HD               # 256 local q features
T = B * L                  # 4096 tokens
NB = 512                   # token block
NT = T // NB               # 8 token blocks
KC = D // 128              # 16 contraction chunks
ROPE_BASE = 10000.0
SCALE = 1.0 / np.sqrt(HD)

_CACHE = {}


def _build_module():
    nc = bacc.Bacc("TRN2", target_bir_lowering=False, debug=False,
                   num_devices=NCORES)

    # host pre-tiles to partition-major [128, ...] so loads are single
    # contiguous 2D DMAs
    d_xT = nc.dram_tensor("xTt", [128, NT * KC * NB], BF16,
                          kind="ExternalInput").ap()
    d_wq = nc.dram_tensor("wq_t", [128, KC * DQ], BF16, kind="ExternalInput").ap()
    d_wkv = nc.dram_tensor("wkv_t", [128, KC * 128], BF16, kind="ExternalInput").ap()
    d_wo = nc.dram_tensor("wo_t", [128, 2 * D], BF16, kind="ExternalInput").ap()
    d_pos = nc.dram_tensor("pos", [1, L], I32, kind="ExternalInput").ap()
    d_invf = nc.dram_tensor("invf", [128, 1], F32, kind="ExternalInput").ap()
    d_ones64 = nc.dram_tensor("ones64", [1, 64], F32R, kind="ExternalInput").ap()
    d_ident = nc.dram_tensor("ident64", [64, 64], BF16, kind="ExternalInput").ap()
    d_tri = nc.dram_tensor("triw", [128, 128], BF16, kind="ExternalInput").ap()
    d_out = nc.dram_tensor("outT", [128, NT, 16, NB], BF16,
                           kind="ExternalOutput").ap()

    with tile.TileContext(nc) as tc, ExitStack() as ctx, \
         nc.allow_low_precision(reason="bf16 matmul pipeline"):
        _kernel(tc, ctx, d_xT, d_wq, d_wkv, d_wo, d_pos, d_invf,
                d_ones64, d_ident, d_tri, d_out)

    nc.compile()
    return nc


def _kernel(tc, ctx, d_xT, d_wq, d_wkv, d_wo, d_pos, d_invf,
            d_ones64, d_ident, d_tri, d_out):
    nc = tc.nc

    wpool = ctx.enter_context(tc.tile_pool(name="weights", bufs=1))
    spool = ctx.enter_context(tc.tile_pool(name="state", bufs=1))

    # ---------------- persistent SBUF tensors ----------------
    wqT = wpool.tile([128, KC * DQ], BF16, tag="wqT")      # 8KB/part
    wkvT = wpool.tile([128, KC * 128], BF16, tag="wkvT")   # 4KB
    woT = wpool.tile([128, 2 * D], BF16, tag="woT")        # 8KB
    nc.sync.dma_start(wqT[:], d_wq[:])
    nc.sync.dma_start(wkvT[:], d_wkv[:])
    nc.sync.dma_start(woT[:], d_wo[:])

    ones64 = wpool.tile([1, 64], F32R, tag="ones64")
    nc.sync.dma_start(ones64[:], d_ones64[:])
    ident64 = wpool.tile([64, 64], BF16, tag="ident64")
    nc.sync.dma_start(ident64[:], d_ident[:])

    # qT: [128, HL/2 * T]; head pair p on partitions (even head rows 0:64,
    # odd head rows 64:128); within a head [even dims | odd dims].
    qT = spool.tile([128, 2, T], BF16, tag="qT")           # 16KB
    # kT duplicated on partitions 64:128 so odd-head matmuls get equal bases.
    kT = spool.tile([128, T], BF16, tag="kT")              # 8KB
    # v token-major bf16, chunk-pair layout: [p, pair, par, 80]
    # col 64 of each chunk = ones (softmax denominator); cols 65:80 pad
    vA = spool.tile([128, 16, 2, 80], BF16, tag="vA")      # 5KB
    nc.gpsimd.memset(vA[:, :, :, :], 1.0)  # ones cols; data cols overwritten
    # attention output, transposed: head pair tiles, b-major columns
    atP = [spool.tile([128, T], BF16, tag=f"atP{p}", name=f"atP{p}") for p in range(2)]  # 16KB

    c128 = spool.tile([128, L], BF16, tag="c128")          # 4KB
    s128 = spool.tile([128, L], BF16, tag="s128")          # 4KB
    # causal triangle window (multiplicative bf16): keep where col >= part
    triW = spool.tile([128, 128], BF16, tag="triW")
    nc.sync.dma_start(triW[:], d_tri[:])

    def build_trig_masks():
        # ---------------- trig tables (cos/sin on 128 partitions) -------------
        # rows 0:32 / 32:64 / 64:96 / 96:128 all hold the same [32] invfreq set,
        # so c128/s128 serve every 32-row band.
        with tc.tile_pool(name="trig", bufs=1) as trig:
            # 5 shared 8KB slots (tags A..E) for all [*, L]-sized temporaries
            pos_i = trig.tile([1, L], I32, tag="sA", name="pos_i")
            nc.sync.dma_start(pos_i[:], d_pos[:])
            pos_f = trig.tile([1, L], F32, tag="sB", name="pos_f")
            nc.vector.tensor_copy(pos_f[:], pos_i[:])
            posb = trig.tile([128, L], F32, tag="sC", name="posb")
            nc.gpsimd.partition_broadcast(posb[:], pos_f[:])
            invf = trig.tile([128, 1], F32, tag="invf")
            nc.sync.dma_start(invf[:], d_invf[:])
            fq = trig.tile([128, L], F32, tag="sD", name="fq")
            nc.vector.tensor_scalar(fq[:], posb[:], invf[:], None, ALU.mult)
            # Cody-Waite range reduction, k via magic-number round-to-nearest
            INV2PI = float(np.float32(1.0 / (2 * np.pi)))
            C1 = float(np.float32(6.28125))
            C2 = float(np.float32(0.0019353071795864769))
            MAGIC = float(np.float32(12582912.0))              # 1.5 * 2**23
            t_a = trig.tile([128, L], F32, tag="sE", name="t_a")
            nc.vector.tensor_scalar(t_a[:], fq[:], INV2PI, None, ALU.mult)
            t_b = trig.tile([128, L], F32, tag="sC", name="t_b")
            nc.vector.tensor_scalar(t_b[:], t_a[:], MAGIC, None, ALU.add)
            kk = trig.tile([128, L], F32, tag="sB", name="kk")
            nc.vector.tensor_scalar(kk[:], t_b[:], MAGIC, None, ALU.subtract)
            nc.vector.tensor_scalar(t_a[:], kk[:], C1, None, ALU.mult)
            nc.vector.tensor_sub(fq[:], fq[:], t_a[:])
            nc.vector.tensor_scalar(t_a[:], kk[:], C2, None, ALU.mult)
            nc.vector.tensor_sub(fq[:], fq[:], t_a[:])         # fq = reduced angle r
            sf = trig.tile([128, L], F32, tag="sB", name="sf")
            nc.scalar.activation(sf[:], fq[:], AF.Sin)
            nc.scalar.activation(t_a[:], fq[:], AF.Abs)
            pi2 = trig.tile([128, 1], F32, tag="pi2")
            nc.gpsimd.memset(pi2[:], float(np.pi / 2))
            cf = trig.tile([128, L], F32, tag="sD", name="cf")
            nc.scalar.activation(cf[:], t_a[:], AF.Sin, bias=pi2[:], scale=-1.0)
            # bake rotation signs into sin table: even-dim bands get -sin
            for band in (0, 2):
                nc.vector.tensor_scalar(sf[band * 32:(band + 1) * 32, :],
                                        sf[band * 32:(band + 1) * 32, :],
                                        -1.0, None, ALU.mult)
            nc.vector.tensor_copy(c128[:], cf[:])
            nc.vector.tensor_copy(s128[:], sf[:])


    # ---------------- phase 1.5: RoPE -------------------------------------
    # per (head-pair, b): bands {ev, od, ev, od} on partitions
    tpool = ctx.enter_context(tc.tile_pool(name="tmp", bufs=1))

    def rope(view, l0):
        # y = x*cos + swap(x)*sgn*sin, swap = exchange 32-row halves per head
        # view(r0, r1) must return the [r0:r1, NB] slice of the target
        u = tpool.tile([128, NB], BF16, tag="ropeU")
        w = tpool.tile([128, NB], BF16, tag="ropeW")
        xsw = tpool.tile([128, NB], BF16, tag="ropeX")
        for band in range(4):
            r0, r1 = band * 32, (band ^ 1) * 32
            nc.vector.tensor_copy(xsw[r0:r0 + 32, :], view(r1, r1 + 32))
        nc.vector.tensor_mul(u[:], view(0, 128), c128[:, l0:l0 + NB])
        nc.vector.tensor_mul(w[:], xsw[:], s128[:, l0:l0 + NB])
        nc.vector.tensor_add(view(0, 128), u[:], w[:])

    def rope_nt(nt):
        b, l0 = nt // 4, (nt % 4) * NB
        c0 = b * L + l0
        for p in range(2):
            rope(lambda r0, r1, p=p: qT[r0:r1, p, c0:c0 + NB], l0)
        rope(lambda r0, r1: kT[r0:r1, c0:c0 + NB], l0)

    # ---------------- phase 1: QKV projections ----------------------------
    with tc.tile_pool(name="xs", bufs=4) as xs, \
         tc.tile_pool(name="vf", bufs=2) as vfp, \
         tc.tile_pool(name="pproj", bufs=2, space="PSUM") as pq, \
         tc.tile_pool(name="pprojkv", bufs=2, space="PSUM") as pkv, \
         tc.tile_pool(name="ptr", bufs=2, space="PSUM") as ptr:
        trig_done = False
        pending_rope = []
        for nt in [0, 1, "trig", 2, 3, 4, 5, 6, 7]:
            if nt == "trig":
                build_trig_masks()
                trig_done = True
                for pnt in pending_rope:
                    rope_nt(pnt)
                pending_rope.clear()
                continue
            # one big DMA for this token block: [128, KC*NB] contiguous
            xk = xs.tile([128, KC * NB], BF16, tag="xk", name=f"xk{nt}")
            nc.sync.dma_start(
                xk[:], d_xT[:, nt * KC * NB:(nt + 1) * KC * NB])
            psq = pq.tile([128, 2, NB], F32, tag="psq", name=f"psq{nt}")
            pskv = pkv.tile([128, NB], F32, tag="pskv", name=f"pskv{nt}")
            for kc in range(KC):
                xkc = xk[:, kc * NB:(kc + 1) * NB]
                st, sp = kc == 0, kc == KC - 1
                for p in range(2):
                    nc.tensor.matmul(
                        psq[:, p, :], wqT[:, kc * DQ + p * 128: kc * DQ + (p + 1) * 128],
                        xkc, start=st, stop=sp, skip_group_check=True)
                nc.tensor.matmul(pskv[:], wkvT[:, kc * 128:(kc + 1) * 128], xkc,
                                 start=st, stop=sp)
            # evictions (ACT copies round fp32 -> bf16); q pair in one op
            nc.scalar.copy(qT[:, :, nt * NB:(nt + 1) * NB], psq[:, :, :])
            nc.scalar.copy(kT[0:64, nt * NB:(nt + 1) * NB], pskv[0:64, :])
            nc.vector.tensor_copy(kT[64:128, nt * NB:(nt + 1) * NB], pskv[0:64, :])
            # V: evict feature-major slab, then PE-transpose to token-major
            vf = vfp.tile([64, NB], BF16, tag="vf", name=f"vf{nt}")
            nc.scalar.copy(vf[:], pskv[64:128, :])
            for c4 in range(4):
                ch = nt * 4 + c4
                psT = ptr.tile([128, 64], BF16, tag="psT", name=f"psT{ch}",
                               padded_shape=[128, 1024])
                nc.tensor.transpose(psT[:], vf[:, c4 * 128:(c4 + 1) * 128],
                                    ident64[:])
                nc.scalar.copy(vA[:, ch // 2, ch % 2, 0:64], psT[:])
            if trig_done:
                rope_nt(nt)
            else:
                pending_rope.append(nt)

    # ---------------- phase 2+3: attention + out-projection ---------------
    # Score chunks processed in PAIRS: one [128, 2, NB] PSUM tile (2 banks),
    # one batched exp -> fp8, one DoubleRow matmul contracting both chunks.
    # pst pool (3 bufs x 2 banks) is shared with the out-projection psums.
    with tc.tile_pool(name="epool", bufs=8) as ep, \
         tc.tile_pool(name="npool", bufs=3) as npool, \
         tc.tile_pool(name="opool", bufs=6) as op, \
         tc.tile_pool(name="pst", bufs=3, space="PSUM") as pst, \
         tc.tile_pool(name="pot", bufs=2, space="PSUM") as pot:
        pending = []  # deferred out-projection emissions (prev block)

        def emit_oproj_pair(nt, mcp):
            po = pst.tile([128, 2, NB], F32, tag="st", name=f"po{nt}_{mcp}")
            for m in range(2):
                mc = 2 * mcp + m
                nc.tensor.matmul(po[:, m, :],
                                 woT[:, 0 * D + mc * 128: 0 * D + (mc + 1) * 128],
                                 atP[0][:, nt * NB:(nt + 1) * NB],
                                 start=True, stop=False, skip_group_check=True)
                nc.tensor.matmul(po[:, m, :],
                                 woT[:, 1 * D + mc * 128: 1 * D + (mc + 1) * 128],
                                 atP[1][:, nt * NB:(nt + 1) * NB],
                                 start=False, stop=True, skip_group_check=True)
            osb = op.tile([128, 2, NB], BF16, tag="osb")
            nc.vector.tensor_copy(osb[:, :, :], po[:, :, :])
            dma_eng = nc.sync if mcp % 2 == 0 else nc.gpsimd
            dma_eng.dma_start(d_out[:, nt, 2 * mcp:2 * mcp + 2, :],
                              osb[:, :, :])

        for b in range(B):
            for ib in range(L // NB):
                # heads interleaved in pairs (even rows 0:64 / odd rows 64:128)
                # so adjacent score matmuls land in different PE row groups
                for p in range(2):
                    qc0 = b * L + ib * NB
                    ot2 = [pot.tile([65, NB], F32, tag="ot", name=f"ot{p}_{o}")
                           for o in range(2)]
                    npc = 2 * (ib + 1)  # chunk pairs
                    for pc in range(npc):
                        st4 = [pst.tile([128, 2, NB], F32, tag="st",
                                        name=f"st{p}_{pc}_{o}") for o in range(2)]
                        for par in range(2):
                            jc = 2 * pc + par
                            c = jc - 4 * ib  # >=0 on diagonal chunks
                            w0 = max(c, 0) * 128  # skip fully-masked q prefix
                            for odd in range(2):
                                rbase = 64 * odd
                                nc.tensor.matmul(
                                    st4[odd][:, par, w0:],
                                    kT[rbase:rbase + 64, b * L + jc * 128: b * L + (jc + 1) * 128],
                                    qT[rbase:rbase + 64, p, qc0 + w0:qc0 + NB],
                                    start=True, stop=True, skip_group_check=True)
                        for odd in range(2):
                            jc0 = 2 * pc
                            cA = jc0 - 4 * ib
                            e_t = ep.tile([128, 2, NB], BF16, tag="e",
                                          name=f"e{p}_{pc}_{odd}")
                            if cA < 0:  # both chunks fully off-diagonal
                                nc.scalar.activation(e_t[:, :, :],
                                                     st4[odd][:, :, :], AF.Exp,
                                                     scale=float(SCALE))
                            else:  # per-chunk exp, skipping masked prefix
                                for par in range(2):
                                    w0 = (cA + par) * 128
                                    nc.scalar.activation(
                                        e_t[:, par, w0:], st4[odd][:, par, w0:],
                                        AF.Exp, scale=float(SCALE))
                            for par in range(2):
                                jc = 2 * pc + par
                                if jc >= 4 * ib:  # triangle window mask
                                    w0 = (jc - 4 * ib) * 128
                                    nc.vector.tensor_mul(
                                        e_t[:, par, w0:w0 + 128],
                                        e_t[:, par, w0:w0 + 128], triW[:])
                            for par in range(2):
                                jc = 2 * pc + par
                                c = jc - 4 * ib
                                w0 = max(c, 0) * 128
                                ch = b * 16 + jc
                                nc.tensor.matmul(
                                    ot2[odd][:, w0:],
                                    vA[:, ch // 2, ch % 2, 0:65],
                                    e_t[:, par, w0:],
                                    start=(jc == 0), stop=(jc == 4 * ib + 3),
                                    skip_group_check=True)
                        # interleave one deferred out-proj pair per pc step
                        if pending:
                            pending.pop(0)()
                    # normalization: denom row -> SBUF (recip_approx misreads
                    # PSUM) -> fast recip -> partition-broadcast on Pool -> scale
                    for odd in range(2):
                        rbase = 64 * odd
                        ot_ps = ot2[odd]
                        # single-op eviction frees the psum bank immediately;
                        # the normalization chain then runs lazily from SBUF
                        ot_sb = npool.tile([65, NB], F32, tag="ot_sb")
                        nc.scalar.copy(ot_sb[:], ot_ps[:])
                        # recip_approx needs a partition-0 SBUF operand
                        dns = npool.tile([1, NB], F32, tag="dns")
                        nc.vector.tensor_copy(dns[:], ot_sb[64:65, :])
                        dnr = npool.tile([1, NB], F32, tag="dnr")
                        nc.vector.reciprocal_approx_fast(dnr[:], dns[:])
                        denbf = npool.tile([64, NB], F32, tag="denbf")
                        nc.gpsimd.partition_broadcast(denbf[:], dnr[:])
                        nc.vector.tensor_mul(
                            atP[p][rbase:rbase + 64, b * L + ib * NB: b * L + (ib + 1) * NB],
                            ot_sb[0:64, :], denbf[:])
                # defer this block's out-projection into the next
                # block's score stream (tail-flushed after the last block)
                nt = b * 4 + ib
                for mcp in range(8):
                    pending.append(
                        lambda nt=nt, mcp=mcp: emit_oproj_pair(nt, mcp))
        for f in pending:  # flush the final block's out-projection
            f()

def _deinterleave_rows(w):
    # [H*64, D] -> per-head rows reordered to [even dims | odd dims]
    h = w.shape[0] // HD
    out = np.empty_like(w)
    perm = np.concatenate([np.arange(0, HD, 2), np.arange(1, HD, 2)])
    for i in range(h):
        out[i * HD:(i + 1) * HD] = w[i * HD:(i + 1) * HD][perm]
    return out


def _part_major(wT, nchunk, m):
    # [nchunk*128, m] -> [128, nchunk*m]: row kc*128+p, col j -> [p, kc*m+j]
    return np.ascontiguousarray(
        wT.reshape(nchunk, 128, m).transpose(1, 0, 2).reshape(128, nchunk * m))


def _prep_inputs(x, pos_ids, wq, wk, wv, wo):
    xT = x.reshape(T, D).T.astype(ml_dtypes.bfloat16)        # [D, T]
    # [128, nt*KC*NB]: block nt = chunks kc of [128, NB]
    xTt = np.ascontiguousarray(
        xT.reshape(KC, 128, NT, NB).transpose(1, 2, 0, 3).reshape(128, -1))
    pos = np.ascontiguousarray(pos_ids.astype(np.int32).reshape(1, L))
    half = HD // 2
    invf = (1.0 / (ROPE_BASE ** (np.arange(half, dtype=np.float32) / half)))
    invf128 = np.ascontiguousarray(np.tile(invf, 4).reshape(128, 1))
    ones64 = np.ones((1, 64), np.float32)
    ident64 = np.eye(64, dtype=ml_dtypes.bfloat16)
    triw = ((np.arange(128)[None, :] >= np.arange(128)[:, None])
            .astype(ml_dtypes.bfloat16))
    in_maps = []
    for c in range(NCORES):
        wq_c = _deinterleave_rows(wq[c * DQ:(c + 1) * DQ])
        wk_c = _deinterleave_rows(wk[c * HD:(c + 1) * HD])
        wv_c = wv[c * HD:(c + 1) * HD]
        wkv_c = np.concatenate([wk_c, wv_c], axis=0)      # [128, D]
        wo_c = wo[:, c * DQ:(c + 1) * DQ]
        in_maps.append({
            "xTt": xTt,
            "wq_t": _part_major(wq_c.T.astype(ml_dtypes.bfloat16), KC, DQ),
            "wkv_t": _part_major(wkv_c.T.astype(ml_dtypes.bfloat16), KC, 128),
            "wo_t": _part_major(wo_c.T.astype(ml_dtypes.bfloat16), 2, D),
            "pos": pos,
            "invf": invf128,
            "ones64": ones64,
            "ident64": ident64,
            "triw": triw,
        })
    return in_maps


def kernel(x, pos_ids, wq, wk, wv, wo, _trace=False):
    x = np.asarray(x)
    if "nc" not in _CACHE:
        _CACHE["nc"] = _build_module()
    nc = _CACHE["nc"]
    in_maps = _prep_inputs(np.asarray(x, np.float32), np.asarray(pos_ids),
                           np.asarray(wq, np.float32), np.asarray(wk, np.float32),
                           np.asarray(wv, np.float32), np.asarray(wo, np.float32))
    res = run_bass_kernel_spmd(nc, in_maps, core_ids=list(range(NCORES)),
                               trace=_trace)
    _CACHE["last_results"] = res
    acc = np.zeros((128, NT, 16, NB), np.float32)
    for r in res.results:
        acc += r["outT"].astype(np.float32)
    # [p, nt, mc, n] -> [ mc*128+p, nt*NB+n ]
    outT = acc.transpose(2, 0, 1, 3).reshape(D, T)
    return np.ascontiguousarray(outT.T).reshape(B, L, D)
